# revision 67
# baseline (speedup 1.0000x reference)
"""Self-contained Trainium2 Bass kernel for causal multi-head attention.

Problem shapes (hardcoded): B=4, T=2048, C=768, H=12, D=64, fp32 I/O.
Sharding: 8 cores = 4 batches x 2 head-groups (6 heads each). Each core
computes its batch's attention for its 6 heads plus the partial output
projection; the host sums the two partial projections per batch.

v2: S matmul in fp8e4 DoubleRow (2x PE rate; qh/kh reshuffled to the
[32, 2, T] k-tile layout by stride-2 partition DMAs), AV matmul flipped to
out[tq=128, dv+1=65] orientation (halves streamed columns and gets the
softmax denominator as one extra column), normalization via per-partition
scalar multiplies, PE transpose-mode matmuls to restore the [dv, t] layout
for the output projection.
"""

import numpy as np
import ml_dtypes
from contextlib import ExitStack

B, T, C, H, D = 4, 2048, 768, 12, 64
NCORE = 8
JG = 384        # head-dim columns per group (6 heads x 64)
NJC = 3         # head-pairs per group (hp), 128 dims each
NCC = 6         # c-chunks of 128 (contraction over C)
NTQ = 4         # tq blocks of 512
NTC = 16        # t chunks of 128
TQB = 512
SCALE = 0.125   # 1/sqrt(64)

_CACHE = {}
last_exec_ns = None
last_results = None


def _build_program(phases=("proj", "attn", "oproj"), bufs=None, legalize=True):
    import concourse.bass as bass
    import concourse.tile as tile
    from concourse import mybir

    dbf = mybir.dt.bfloat16
    df8 = mybir.dt.float8e4
    df32 = mybir.dt.float32
    AF = mybir.ActivationFunctionType
    DR = mybir.MatmulPerfMode.DoubleRow

    nc = bass.Bass("TRN2", target_bir_lowering=False, debug=False)
    qT_d = nc.dram_tensor("qT", [C, T], dbf, kind="ExternalInput").ap()
    kT_d = nc.dram_tensor("kT", [C, T], dbf, kind="ExternalInput").ap()
    vT_d = nc.dram_tensor("vT", [C, T], dbf, kind="ExternalInput").ap()
    wq_d = nc.dram_tensor("wqT", [C, JG], dbf, kind="ExternalInput").ap()
    wk_d = nc.dram_tensor("wkT", [C, JG], dbf, kind="ExternalInput").ap()
    wv_d = nc.dram_tensor("wvT", [C, JG], dbf, kind="ExternalInput").ap()
    wo_d = nc.dram_tensor("woT", [JG, C], dbf, kind="ExternalInput").ap()
    out_d = nc.dram_tensor("out", [T, C], df32, kind="ExternalOutput").ap()

    bf = dict(ps_s=2, ps_pr=2, ps_y=2, pp=12, outp=8)
    bf.update(bufs or {})
    with tile.TileContext(nc) as tc, ExitStack() as ctx:
        pers = ctx.enter_context(tc.tile_pool(name="pers", bufs=1))
        ps_s = ctx.enter_context(tc.tile_pool(name="ps_s", bufs=bf["ps_s"], space="PSUM"))
        ps_pr = ctx.enter_context(tc.tile_pool(name="ps_pr", bufs=bf["ps_pr"], space="PSUM"))
        ps_y = ctx.enter_context(tc.tile_pool(name="ps_y", bufs=bf["ps_y"], space="PSUM"))
        pp = ctx.enter_context(tc.tile_pool(name="pp", bufs=bf["pp"]))
        rtp = ctx.enter_context(tc.tile_pool(name="rtp", bufs=4))
        ytfp = ctx.enter_context(tc.tile_pool(name="ytfp", bufs=2))
        outp = ctx.enter_context(tc.tile_pool(name="outp", bufs=bf["outp"]))

        def ptile(shape, dtype, tag):
            return pers.tile(shape, dtype, tag=tag, name=tag)

        # One wide SBUF tile per DMA-loaded tensor (chunk i at cols [i*W,
        # (i+1)*W)): a whole column-wave loads in a single strided DMA, so
        # per-transfer overhead is paid once instead of NCC times.
        wq_big = ptile([128, NCC * JG], dbf, "wqbig")
        wk_big = ptile([128, NCC * JG], dbf, "wkbig")
        wv_big = ptile([128, NCC * JG], dbf, "wvbig")
        wo_big = ptile([128, NJC * C], dbf, "wobig")
        qT_big = ptile([128, NCC * T], dbf, "qTbig")
        kT_big = ptile([128, NCC * T], dbf, "kTbig")
        vT_big = ptile([128, NCC * T], dbf, "vTbig")
        wq_sb = [wq_big[:, i * JG:(i + 1) * JG] for i in range(NCC)]
        wk_sb = [wk_big[:, i * JG:(i + 1) * JG] for i in range(NCC)]
        wv_sb = [wv_big[:, i * JG:(i + 1) * JG] for i in range(NCC)]
        wo_sb = [wo_big[:, i * C:(i + 1) * C] for i in range(NJC)]
        qT_sb = [qT_big[:, i * T:(i + 1) * T] for i in range(NCC)]
        kT_sb = [kT_big[:, i * T:(i + 1) * T] for i in range(NCC)]
        vT_sb = [vT_big[:, i * T:(i + 1) * T] for i in range(NCC)]

        # fp8 staging (proj output) and DoubleRow-layout tiles per head-pair.
        # One combined tensor (sliced) so CoreSim's conservative span checks
        # on the stride-2-partition reshuffle reads stay within one tensor.
        qk8_big = ptile([128, 2 * NJC * T], df8, "qk8big")
        q8_sb = [qk8_big[:, i * T:(i + 1) * T] for i in range(NJC)]
        k8_sb = [qk8_big[:, (NJC + i) * T:(NJC + i + 1) * T] for i in range(NJC)]
        # query-block 0 computes S in bf16 straight from these copies, so no
        # reshuffle DMA sits on the kernel-prefix critical path
        qh_bf = [ptile([128, TQB], dbf, f"qbf{i}") for i in range(NJC)]
        kh_bf = [ptile([128, TQB], dbf, f"kbf{i}") for i in range(NJC)]
        # [64, 2, T]: partition r=ab*32+q holds src partitions 2r+j in block j
        qdr = [ptile([64, 2, T], df8, f"qdr{i}") for i in range(NJC)]
        kdr = [ptile([64, 2, T], df8, f"kdr{i}") for i in range(NJC)]

        va_sb = [ptile([128, 6 * 65], dbf, f"va{i}") for i in range(NTC)]
        yt_sb = [ptile([128, T], dbf, f"yt{i}") for i in range(NJC)]
        idt = ptile([128, 128], dbf, "idt")

        # DMA order matters: the minimal prefix for starting attention is
        # weights + the first 512-column block of qT/kT/vT. Later column
        # blocks are slot-scheduled through the spread queue so the q/k
        # reshuffle DMAs (which gate the first S matmuls) are not stuck
        # behind 30us of input waves on the serial DMA engines.
        def wave(big, dr, nch, width, lo, hi):
            dst = big[:].rearrange("p (i c) -> p i c", i=nch)[:, :, lo:hi]
            src = dr.rearrange("(i p) c -> p i c", p=128)[:, :, lo:hi]
            nc.sync.dma_start(dst, src)

        wave(wq_big, wq_d, NCC, JG, 0, JG)
        wave(qT_big, qT_d, NCC, T, 0, TQB)
        wave(wk_big, wk_d, NCC, JG, 0, JG)
        wave(kT_big, kT_d, NCC, T, 0, TQB)
        wave(wv_big, wv_d, NCC, JG, 0, JG)

        def wave_g(big, dr, b):
            return lambda: wave(big, dr, NCC, T, b * TQB, (b + 1) * TQB)

        # ones column of va (col 64 of each per-head 65-block): the flipped
        # AV matmul then emits the softmax denominator as output column 64.
        for t in range(NTC):
            vav = va_sb[t][:].rearrange("p (h x) -> p h x", h=6)
            nc.gpsimd.memset(vav[:, :, 64:65], 1.0)

        # initialize staging in full so conservative span checks never see
        # uninitialized bytes (cols 0..512 of q8 are never written otherwise)
        nc.gpsimd.memset(qk8_big[:], 0.0)

        # causal triangle mask for the last block's diagonal chunks (the
        # Pool affine_select chain would otherwise gate the kernel tail)
        tri = ptile([128, 2, 128], dbf, "tri")
        nc.gpsimd.memset(tri[:], 1.0)
        nc.gpsimd.affine_select(
            out=tri[:], in_=tri[:], pattern=[[0, 2], [1, 128]],
            compare_op=mybir.AluOpType.is_ge, fill=0.0,
            base=0, channel_multiplier=-1)

        # identity for PE transpose-mode matmuls
        nc.gpsimd.memset(idt[:], 1.0)
        nc.gpsimd.affine_select(
            out=idt[:], in_=idt[:], pattern=[[1, 128]],
            compare_op=mybir.AluOpType.is_equal, fill=0.0,
            base=0, channel_multiplier=-1)

        # PE warm-up during the DMA prefix: ~4us of dummy matmuls releases
        # the HAM clock gate (1.2 -> 2.4 GHz) before the real work arrives.
        warm = pers.tile([128, 64], dbf, tag="warm", name="warm")
        nc.vector.memset(warm[:], 0.0)
        wps = ps_pr.tile([64, 64], df32, tag="proj", name="warm_ps")
        for i in range(6):
            nc.tensor.matmul(wps[:], lhsT=warm[:, 0:64], rhs=warm[:],
                             start=True, stop=True)
        # touch Exp once so the ACT table set loads during the DMA prefix
        wexp = pers.tile([1, 64], dbf, tag="wexp", name="wexp")
        nc.scalar.activation(wexp[:], warm[0:1, :], AF.Exp, scale=SCALE)


        # --- projection emitters (used up-front and as PE filler work that
        # hides inside the ACT-bound attention stream)
        qk_ps = {}

        def qk_part(src, w, st8, ddr, jc, tb, label, half):
            key = (label, jc, tb)
            if half == 0:
                qk_ps[key] = ps_pr.tile([128, TQB], df32, tag="proj",
                                        name=f"ps_{label}_{jc}_{tb}")
            ps = qk_ps[key]
            for cc in range(2 * half, 2 * half + 2):
                nc.tensor.matmul(ps[:],
                                 lhsT=w[cc][:, jc * 128:(jc + 1) * 128],
                                 rhs=src[cc][:, tb * TQB:(tb + 1) * TQB],
                                 start=(cc == 0), stop=(cc == NCC - 1))
            if half == 2:
                del qk_ps[key]
                sl = slice(tb * TQB, (tb + 1) * TQB)
                if tb == 0:
                    bf_dst = qh_bf if label == "qh" else kh_bf
                    nc.vector.tensor_copy(bf_dst[jc][:], ps[:])
                    return
                nc.vector.tensor_copy(st8[jc][:, sl], ps[:])
                # reshuffle to DoubleRow layout: per (ab, j) contiguous
                # 32-partition block; d = ab*64 + j*32 + k on both sides
                for ab in range(2):
                    for j in range(2):
                        p0 = ab * 64 + j * 32
                        nc.sync.dma_start(
                            ddr[jc][ab * 32:(ab + 1) * 32, j, sl],
                            st8[jc][p0:p0 + 32, sl])

        def k8_from_bf(jc):
            # deferred fp8 conversion + reshuffle of the tb0 key columns
            # (needed by every later query block at kc 0..3)
            def emit():
                sl = slice(0, TQB)
                nc.vector.tensor_copy(k8_sb[jc][:, sl], kh_bf[jc][:])
                for ab in range(2):
                    for j in range(2):
                        p0 = ab * 64 + j * 32
                        nc.sync.dma_start(
                            kdr[jc][ab * 32:(ab + 1) * 32, j, sl],
                            k8_sb[jc][p0:p0 + 32, sl])
            return emit

        def qk_group(src, w, st8, ddr, jc, tb, label):
            for h in range(3):
                qk_part(src, w, st8, ddr, jc, tb, label, h)

        v_ps = {}

        def v_part(t, half):
            if half == 0:
                v_ps[t] = ps_pr.tile([128, JG], df32, tag="proj", name=f"psv_{t}")
            ps = v_ps[t]
            for cc in range(2 * half, 2 * half + 2):
                nc.tensor.matmul(ps[:], lhsT=vT_sb[cc][:, t * 128:(t + 1) * 128],
                                 rhs=wv_sb[cc][:],
                                 start=(cc == 0), stop=(cc == NCC - 1))
            if half == 2:
                del v_ps[t]
                vav = va_sb[t][:].rearrange("p (h x) -> p h x", h=6)
                psv = ps[:].rearrange("p (h d) -> p h d", h=6)
                nc.vector.tensor_copy(vav[:, :, 0:64], psv[:])

        def v_group(t):
            for h in range(3):
                v_part(t, h)

        def o_half(t, nb, ot, act_copy=False, pool=None):
            ps = (pool or ps_pr).tile([128, 384], df32,
                                      tag="proj" if pool is None else "s",
                                      name=f"pso_{t}_{nb}")
            for jc in range(NJC):
                nc.tensor.matmul(ps[:], lhsT=yt_sb[jc][:, t * 128:(t + 1) * 128],
                                 rhs=wo_sb[jc][:, nb * 384:(nb + 1) * 384],
                                 start=(jc == 0), stop=(jc == NJC - 1))
            if act_copy:
                nc.scalar.copy(ot[:, nb * 384:(nb + 1) * 384], ps[:])
            else:
                nc.vector.tensor_copy(ot[:, nb * 384:(nb + 1) * 384], ps[:])

        def o_group(t):
            ot = outp.tile([128, C], df32, tag="o", name=f"o_{t}")
            for nb in range(2):
                o_half(t, nb, ot)
            nc.sync.dma_start(out_d[t * 128:(t + 1) * 128, :], ot[:])

        # Up-front: everything attention needs for query-block 0: all three
        # head-pairs' q/k tb0 groups (their reshuffle chains gate each hp's
        # first S matmul) plus the first four va chunks. The vT0 wave is
        # issued after the jc0 reshuffles so they aren't stuck behind it on
        # the serial DMA engines; the v groups (gated on vT0) are emitted
        # after the first S so they don't delay it in the PE stream.
        for jc in range(NJC):
            qk_group(qT_sb, wq_sb, q8_sb, qdr, jc, 0, "qh")
        qk_group(kT_sb, wk_sb, k8_sb, kdr, 0, 0, "kh")
        wave(vT_big, vT_d, NCC, T, 0, TQB)

        skip_attn = "attn" not in phases
        skip_oproj = "oproj" not in phases

        def kh_g(jc, tb, half):
            return lambda: qk_part(kT_sb, wk_sb, k8_sb, kdr, jc, tb, "kh", half)

        def qh_g(jc, tb, half):
            return lambda: qk_part(qT_sb, wq_sb, q8_sb, qdr, jc, tb, "qh", half)

        def v_g(t, half):
            return lambda: v_part(t, half)

        # Blocks run qb-major: all three head pairs of query-block 0 first,
        # then query-block 1, etc. Each qb round depends only on input column
        # waves <= qb, so early rounds fully overlap the later DMA waves (and
        # output rows complete early and stream out through the whole kernel).
        blocks = [(qb, hp) for qb in range(NTQ) for hp in range(NJC)]
        nkc_of = {(qb, hp): 4 * qb + 4 for (qb, hp) in blocks}
        blk_start = {}
        acc = 0
        for b in blocks:
            blk_start[b] = acc
            acc += nkc_of[b]

        def to_slot(qb, hp, kc):
            return blk_start[(qb, hp)] + kc

        # Remaining projection groups in a first-use-ordered queue. One group
        # is emitted per kc iteration so PE filler work never monopolizes the
        # stream. The q/k groups feed a copy + reshuffle-DMA chain (~4.5us)
        # before the S matmuls can consume them, so they hoist further.
        # Input waves ("dma" kind, no PE work) are slot-scheduled too, placed
        # so each is complete just before the proj groups that consume it.
        HOIST_QK = 12
        HOIST = 3
        HOIST_WAVE = 12
        # entries: (slot, deadline, kind, fn). `slot` is when we'd like to
        # emit; `deadline` is the step by which the entry MUST be emitted
        # (its reader is emitted that step) regardless of the PE throttle.
        spread = []
        for jc in range(NJC):
            for tb in range(1, NTQ):
                # qh[jc][tb] first used at block (tb, jc) kc=0 (S emitted 2
                # steps earlier); kh[jc][tb] at block (tb, jc) kc=4*tb
                s_q = to_slot(tb, jc, 0) - HOIST_QK
                d_q = to_slot(tb, jc, 0) - 3
                s_k = to_slot(tb, jc, 4 * tb) - HOIST_QK
                d_k = to_slot(tb, jc, 4 * tb) - 3
                for h in range(3):
                    spread.append((s_q - 0.4 * (2 - h), d_q, "pe", qh_g(jc, tb, h)))
                    spread.append((s_k - 0.4 * (2 - h), d_k, "pe", kh_g(jc, tb, h)))
        for t in range(1, NTC):
            # va[t] is read by AV at step kc=t of block (t//4, 0); the write
            # must be EMITTED before that step or the dep tracker misses it.
            s_v = (t - 0.75) if t < 4 else (to_slot(t // 4, 0, t) - HOIST)
            d_v = to_slot(t // 4, 0, t) - 0.5
            for h in range(3):
                spread.append((s_v - 0.3 * (2 - h), d_v, "pe", v_g(t, h)))
        for jc in range(NJC):
            # kdr tb0 chunks first needed by block (1, jc) at kc=0
            spread.append((to_slot(1, jc, 0) - HOIST_QK,
                           to_slot(1, jc, 0) - 3, "pe", k8_from_bf(jc)))
        for b in range(1, NTQ):
            s0 = to_slot(b, 0, 0) - HOIST_QK
            spread.append((s0 - HOIST_WAVE, s0 - 4, "dma", wave_g(qT_big, qT_d, b)))
            spread.append((s0 - HOIST_WAVE + 2, s0 - 2, "dma", wave_g(kT_big, kT_d, b)))
            spread.append((s0 - HOIST_WAVE + 4, s0, "dma", wave_g(vT_big, vT_d, b)))
        spread.append((6, 10, "dma", lambda: wave(wo_big, wo_d, NJC, C, 0, C)))
        o_start = max(s for s, d, kind, _ in spread if kind == "dma")
        spread.sort(key=lambda e: e[0])
        spread.reverse()  # pop from the end

        if skip_attn:
            while spread:
                spread.pop()[-1]()

        def emit_due(slot):
            pe_done = 0
            keep = []
            while spread and spread[-1][0] <= slot:
                ent = spread.pop()
                if ent[2] == "pe" and pe_done >= 2 and ent[1] > slot:
                    keep.append(ent)  # due but deferrable
                    continue
                if ent[2] == "pe":
                    pe_done += 1
                ent[-1]()
            spread.extend(reversed(keep))

        # Attention: S.T tiles [tk=128, tq<=512] via fp8 DoubleRow, exp on
        # ScalarE, causal mask via affine_select on GPSIMD, then flipped AV
        # (out [tq=128, 65]) accumulating y + denominator per tq chunk.
        #
        # Software-pipelined: the S matmul for step i+1 is emitted BEFORE the
        # AV matmuls of step i (including across block boundaries), so ACT
        # always has the next S tile ready and runs exp back-to-back; PE's
        # AV/S work hides under the exp stream.
        steps = [(qb, hp, kc) for (qb, hp) in blocks
                 for kc in range(nkc_of[(qb, hp)])]
        yps_of = {}
        spair_of = {}

        def emit_s(i):
            qb, hp, kc = steps[i]
            if kc == 0:
                yps_of[(qb, hp)] = [
                    ps_y.tile([128, 4, 65], df32, tag="y",
                              name=f"y_{hp}_{qb}_{ab}") for ab in range(2)]
            off = max(0, kc * 128 - qb * TQB)
            spair = ps_s.tile([128, 1024], df32, tag="s",
                              name=f"s_{hp}_{qb}_{kc}")
            spair_of[i] = spair
            for ab in range(2):
                if qb == 0:
                    nc.tensor.matmul(
                        spair[:, ab * TQB + off: (ab + 1) * TQB],
                        lhsT=kh_bf[hp][ab * 64:(ab + 1) * 64, kc * 128:(kc + 1) * 128],
                        rhs=qh_bf[hp][ab * 64:(ab + 1) * 64, off:TQB],
                        start=True, stop=True)
                else:
                    nc.tensor.matmul(
                        spair[:, ab * TQB + off: (ab + 1) * TQB],
                        lhsT=kdr[hp][ab * 32:(ab + 1) * 32, :, kc * 128:(kc + 1) * 128],
                        rhs=qdr[hp][ab * 32:(ab + 1) * 32, :, qb * TQB + off:(qb + 1) * TQB],
                        start=True, stop=True, perf_mode=DR)

        def norm_block(qb, hp, yps):
            if True:
                # block done: normalize (denominator is column 64 of yps),
                # transpose back to [dv, t] layout, and stage for o-proj.
                # tl-granular so the final block's output rows pipeline:
                # mul(tl) -> transpose(tl) -> yt copy(tl) -> o_group inline.
                last_block = (qb, hp) == blocks[-1]
                pt = ps_pr.tile([128, 4, 128], dbf, tag="proj", name=f"pt_{hp}_{qb}")
                ytf = ytfp.tile([128, 4, 128], dbf, tag="ytf",
                                name=f"ytf_{hp}_{qb}")
                rts = []
                for ab in range(2):
                    rt = rtp.tile([128, 4], df32, tag="rt",
                                  name=f"rt_{hp}_{qb}_{ab}")
                    nc.vector.reciprocal(rt[:], yps[ab][:, :, 64])
                    rts.append(rt)
                ots = {}
                for tl in range(4):
                    for ab in range(2):
                        nc.vector.tensor_scalar_mul(
                            ytf[:, tl, ab * 64:(ab + 1) * 64],
                            yps[ab][:, tl, 0:64], rts[ab][:, tl:tl + 1])
                    nc.tensor.transpose(pt[:, tl, :], ytf[:, tl, :], idt[:])
                    nc.vector.tensor_copy(
                        yt_sb[hp][:, qb * TQB + tl * 128:qb * TQB + (tl + 1) * 128],
                        pt[:, tl, :])
                    if last_block and not skip_oproj:
                        # pipeline: o(t) half-groups interleave across tl so
                        # PE / ACT / DVE / DMA all stream during the tail
                        t = 4 * qb + tl
                        ots[t] = outp.tile([128, C], df32, tag="o", name=f"o_{t}")
                        o_half(t, 0, ots[t], act_copy=True)
                        if tl > 0:
                            tp = t - 1
                            o_half(tp, 1, ots[tp])
                            nc.sync.dma_start(
                                out_d[tp * 128:(tp + 1) * 128, :], ots[tp][:])
                        if tl == 3:
                            o_half(t, 1, ots[t])
                            nc.sync.dma_start(
                                out_d[t * 128:(t + 1) * 128, :], ots[t][:])
                if hp == NJC - 1 and not last_block and not skip_oproj:
                    # all head pairs done for this qb: queue its output rows
                    oq.extend((t, nb) for t in range(4 * qb, 4 * qb + 4)
                              for nb in range(2))


        oq = []  # deferred output-projection half-chunks (t, nb)
        o_cur = {}
        pending_norm = []
        if not skip_attn:
            for j in range(4):
                emit_s(j)
        for jc in range(1, NJC):
            qk_group(kT_sb, wk_sb, k8_sb, kdr, jc, 0, "kh")
        v_group(0)
        if skip_attn:
            for t in range(1, 4):
                v_group(t)
        for i in range(len(steps) if not skip_attn else 0):
            qb, hp, kc = steps[i]
            nkc = nkc_of[(qb, hp)]
            if i + 4 < len(steps):
                emit_s(i + 4)
            emit_due(to_slot(qb, hp, kc))
            off = max(0, kc * 128 - qb * TQB)
            spair = spair_of.pop(i)
            yps = yps_of[(qb, hp)]
            ppair = pp.tile([128, 1024], dbf, tag="p",
                            name=f"p_{hp}_{qb}_{kc}")
            sview = spair[:].rearrange("p (a x) -> p a x", a=2)
            pview = ppair[:].rearrange("p (a x) -> p a x", a=2)
            nc.scalar.activation(pview[:, :, off:TQB], sview[:, :, off:TQB],
                                 AF.Exp, scale=SCALE)
            if kc >= 4 * qb:  # diagonal chunk: zero tq < tk entries.
                # Only columns [off, off+128) can violate causality (tq and
                # tk in the same 128-chunk); the rest of the exp'd region is
                # strictly above the diagonal, so the mask is 128 wide.
                # base = tq_global - tk_global = qb*TQB + off - kc*128 = 0
                # always on the diagonal, so the precomputed tri mask works.
                if (qb, hp) == blocks[-1]:
                    nc.vector.tensor_mul(
                        pview[:, :, off:off + 128],
                        pview[:, :, off:off + 128], tri[:])
                else:
                    nc.gpsimd.affine_select(
                        out=pview[:, :, off:off + 128], in_=pview[:, :, off:off + 128],
                        pattern=[[0, 2], [1, 128]],
                        compare_op=mybir.AluOpType.is_ge, fill=0.0,
                        base=qb * TQB + off - kc * 128, channel_multiplier=-1)
            if kc == 0 and pending_norm:
                norm_block(*pending_norm.pop(0))
            for ab in range(2):
                hl = 2 * hp + ab
                var = va_sb[kc][:].rearrange("p (h x) -> p h x", h=6)
                for tg in range(max(kc, 4 * qb), 4 * qb + 4):
                    tl = tg - 4 * qb
                    # One accumulation group per (ab, block): start pending-
                    # zeroes the whole bank, each tl's first touch overwrites
                    # via the per-byte pending-zero bits, stop on the last AV.
                    nc.tensor.matmul(
                        yps[ab][:, tl, :],
                        lhsT=pview[:, ab, tl * 128:(tl + 1) * 128],
                        rhs=var[:, hl, :],
                        start=(kc == 0 and tl == 0), stop=(kc == nkc - 1),
                        skip_group_check=True)
            if (qb, hp) == blocks[-1] and kc >= 4 * qb and not skip_oproj:
                # final block: as soon as tl's diagonal AV lands, run its
                # norm -> transpose -> yt copy -> o chain under the
                # remaining diagonal exps
                tl = kc - 4 * qb
                if tl == 0:
                    tail_pt = ps_pr.tile([128, 4, 128], dbf, tag="proj",
                                         name="pt_tail")
                    tail_ytf = ytfp.tile([128, 4, 128], dbf, tag="ytf",
                                         name="ytf_tail")
                    tail_ots = {}
                for ab in range(2):
                    rt = rtp.tile([128, 1], df32, tag="rt1",
                                  name=f"rtt_{tl}_{ab}")
                    nc.vector.reciprocal(rt[:], yps[ab][:, tl, 64:65])
                    nc.vector.tensor_scalar_mul(
                        tail_ytf[:, tl, ab * 64:(ab + 1) * 64],
                        yps[ab][:, tl, 0:64], rt[:])
                nc.tensor.transpose(tail_pt[:, tl, :], tail_ytf[:, tl, :], idt[:])
                nc.vector.tensor_copy(
                    yt_sb[hp][:, qb * TQB + tl * 128:qb * TQB + (tl + 1) * 128],
                    tail_pt[:, tl, :])
                t = 4 * qb + tl
                pl = ps_s if tl >= 2 else None
                tail_ots[t] = outp.tile([128, C], df32, tag="o", name=f"o_{t}")
                o_half(t, 0, tail_ots[t], act_copy=True, pool=pl)
                nc.sync.dma_start(out_d[t * 128:(t + 1) * 128, 0:384],
                                  tail_ots[t][:, 0:384])
                if tl > 0:
                    tp = t - 1
                    o_half(tp, 1, tail_ots[tp], pool=pl)
                    nc.sync.dma_start(out_d[tp * 128:(tp + 1) * 128, 384:C],
                                      tail_ots[tp][:, 384:C])
                if tl == 3:
                    o_half(t, 1, tail_ots[t], act_copy=True, pool=pl)
                    nc.sync.dma_start(out_d[t * 128:(t + 1) * 128, 384:C],
                                      tail_ots[t][:, 384:C])
            slot = to_slot(qb, hp, kc)
            if oq and slot >= o_start and (not spread or spread[-1][0] > slot + 1):
                t, nb = oq[0]
                if nb == 0:
                    o_cur[t] = outp.tile([128, C], df32, tag="o", name=f"o_{t}")
                o_half(t, nb, o_cur[t])
                if nb == 1:
                    nc.sync.dma_start(out_d[t * 128:(t + 1) * 128, :],
                                      o_cur.pop(t)[:])
                oq.pop(0)
            if kc == nkc - 1:
                if (qb, hp) != blocks[-1] or skip_oproj:
                    pending_norm.append((qb, hp, yps))
                del yps_of[(qb, hp)]

        while pending_norm:
            norm_block(*pending_norm.pop(0))

        # Output projection tail (whatever was not absorbed into attention)
        if not skip_oproj and not skip_attn:
            for (t, nb) in oq:
                if nb == 0:
                    o_cur[t] = outp.tile([128, C], df32, tag="o", name=f"o_{t}")
                o_half(t, nb, o_cur[t])
                if nb == 1:
                    nc.sync.dma_start(out_d[t * 128:(t + 1) * 128, :],
                                      o_cur.pop(t)[:])

    if legalize:
        _legalize_mm_waits(nc, mybir)
    return nc


def _legalize_mm_waits(nc, mybir):
    """TRN2 Matmult/Ldweights support a single sync-wait slot; walrus errors
    on more. Hoist excess waits onto same-engine NoOps inserted just before
    the offending instruction (engine-sequential, so semantics unchanged)."""
    nop_i = 0
    for f in nc.m.functions:
        for blk in f.blocks:
            insts = blk.instructions
            new = []
            last_by_eng = {}   # engine -> index in `new` of its last instruction
            changed = False
            for inst in insts:
                si = getattr(inst, "sync_info", None)
                waits = list(si.on_wait) if si and si.on_wait else []
                if len(waits) > 1:
                    rest = waits[:-1]
                    # Absorb one excess wait into the immediately preceding
                    # same-engine instruction when it carries no waits and no
                    # updates: waiting earlier on the same queue is semantics-
                    # preserving, and with no on_update no other engine can
                    # observe the delay.
                    pi = last_by_eng.get(str(inst.engine))
                    if pi is not None and rest:
                        prev = new[pi]
                        psi = getattr(prev, "sync_info", None)
                        if (type(prev).__name__ not in
                                ("InstNoOp", "InstDrain", "InstEventSemaphore",
                                 "InstUnconditionalBranch", "InstCompareAndBranch")
                                and (psi is None or
                                     (not psi.on_wait and not psi.on_update))):
                            prev.sync_info = mybir.SyncInfo(
                                on_wait=[rest.pop(0)],
                                on_update=list(psi.on_update) if psi else [])
                    for w in rest:
                        nop = mybir.InstNoOp(name=f"mmwait-{nop_i}", ins=[], outs=[])
                        nop_i += 1
                        nop.engine = inst.engine
                        nop.sync_info = mybir.SyncInfo(on_wait=[w], on_update=[])
                        new.append(nop)
                    si.on_wait = [waits[-1]]
                    changed = True
                last_by_eng[str(inst.engine)] = len(new)
                new.append(inst)
            if changed:
                blk.instructions = new


def get_program():
    if "nc" not in _CACHE:
        _CACHE["nc"] = _build_program()
    return _CACHE["nc"]


def make_in_maps(q, k, v, w_q, w_k, w_v, w_o):
    bf = ml_dtypes.bfloat16
    in_maps = []
    for core in range(NCORE):
        b, g = divmod(core, 2)
        js = slice(g * JG, (g + 1) * JG)
        in_maps.append({
            "qT": np.asarray(q[b]).T.astype(bf),
            "kT": np.asarray(k[b]).T.astype(bf),
            "vT": np.asarray(v[b]).T.astype(bf),
            "wqT": np.asarray(w_q)[js, :].T.astype(bf),
            "wkT": np.asarray(w_k)[js, :].T.astype(bf),
            "wvT": np.asarray(w_v)[js, :].T.astype(bf),
            "woT": np.asarray(w_o)[:, js].T.astype(bf),
        })
    return in_maps


def kernel(q, k, v, w_q, w_k, w_v, w_o, trace=False):
    global last_exec_ns, last_results
    from concourse import bass_utils

    nc = get_program()
    in_maps = make_in_maps(q, k, v, w_q, w_k, w_v, w_o)
    try:
        res = bass_utils.run_bass_kernel_spmd(
            nc, in_maps, core_ids=list(range(NCORE)), trace=trace)
    except ModuleNotFoundError:
        # axon client without the NTFF profiling hook: run untraced
        res = bass_utils.run_bass_kernel_spmd(
            nc, in_maps, core_ids=list(range(NCORE)), trace=False)
    last_exec_ns = res.exec_time_ns
    last_results = res
    out = np.zeros((B, T, C), np.float32)
    for core in range(NCORE):
        b = core // 2
        out[b] += res.results[core]["out"]
    return out
